# revision 1
# baseline (speedup 1.0000x reference)
"""Trainium2 Bass kernel: Deepseek-style decoder layer (dense transformer),
tensor-parallel over 8 NeuronCores.

Per core: 2 Q heads + their GQA KV head; attention computed in transposed
(scoresT) layout so softmax-denominators come from ones-matmuls and no probs
transpose is needed. Attention head outputs are AllToAll'd so each core gets
all heads for its 256-token sequence shard; o_proj/residual/norm2 run
seq-sharded; h2 shards are AllGathered for the tensor-parallel gate/up
matmuls (1024 FF cols/core); gated activations are AllToAll'd back to
seq-shards for the full down_proj. RMSNorm weights fold into following
projections on host; rstd row-scales fold into RoPE multipliers/epilogues.
All matmuls fp16 (full PE rate), fp32 PSUM accumulation + fp32 residuals.
"""
import sys
import os
import numpy as np

for _p in ("/opt/trn_rl_repo", "/root/.axon_site/_ro/trn_rl_repo"):
    if os.path.isdir(_p) and _p not in sys.path:
        sys.path.append(_p)

B, S, D = 1, 2048, 2048
H, KVH, HD = 16, 4, 128
FF = 8192
EPS = 1e-6
THETA = 10000.0
NC = 8
P = 128
SBLK = 512
NSB = S // SBLK          # 4 s-blocks
DCH = D // P             # 16 d-chunks
SHARD = S // NC          # 256 tokens per core
FFSH = FF // NC          # 1024
NKC = S // P             # 16 sk-chunks
EXP_SHIFT = -8.0         # exp(x+shift): cancels in softmax ratio, keeps fp16
                         # probs far from overflow without a max pass

_CACHE = {}


def _install_drain_patch(tile_mod, mybir):
    """Stock tail-drain puts one wait per outstanding proc on a single sync-
    queue CTRL op, which supports only ONE wait -> chain single-wait drains."""
    from concourse.vector_clock import ScopedClock

    def _split(self, tick_clock, wait_clock):
        nc = self.nc
        drain_inst = nc.sync.drain()
        wait_clock.add_sem_waits(
            drain_inst.ins, ScopedClock({None: tick_clock.global_clock}))
        si = drain_inst.ins.sync_info
        if si is not None and len(si.on_wait) > 1:
            waits, upd = list(si.on_wait), list(si.on_update)
            drain_inst.ins.sync_info = mybir.SyncInfo(
                on_wait=waits[:1], on_update=[])
            rest = waits[1:]
            while rest:
                chunk, rest = rest[:1], rest[1:]
                extra = nc.sync.drain()
                extra.ins.sync_info = mybir.SyncInfo(
                    on_wait=chunk, on_update=([] if rest else upd))
        nc.all_engine_barrier()
        assert self.sems is not None
        popped = nc._tile_sem_poison_stack.pop()
        assert popped is self._sem_poison
        nc.clear_and_free_semaphores(list(self.sems.allocated().values()))
        nc.all_engine_barrier()

    tile_mod.TileContext._drain_and_barrier = _split


def _split_waits(nc, mybir):
    """HW allows one sync-wait per instruction on these queues; Tile can emit
    several at dependency joins. Insert same-engine no-ops, each carrying one
    excess wait, immediately before the offending instruction."""
    eng_map = {
        mybir.EngineType.DVE: nc.vector,
        mybir.EngineType.Activation: nc.scalar,
        mybir.EngineType.PE: nc.tensor,
        mybir.EngineType.Pool: nc.gpsimd,
        mybir.EngineType.SP: nc.sync,
    }
    for bb in nc.main_func.blocks:
        todo = []
        for inst in bb.instructions:
            si = getattr(inst, "sync_info", None)
            if si is not None and len(si.on_wait) > 1:
                todo.append(inst)
        if not todo:
            continue
        inserts = {}
        created = []
        for inst in todo:
            si = inst.sync_info
            waits = list(si.on_wait)
            nops = []
            for w in waits[:-1]:
                nop = eng_map[inst.engine].nop().ins
                nop.sync_info = mybir.SyncInfo(on_wait=[w], on_update=[])
                nops.append(nop)
                created.append(nop)
            inst.sync_info = mybir.SyncInfo(
                on_wait=[waits[-1]], on_update=list(si.on_update))
            inserts[id(inst)] = nops
        created_ids = {id(n) for n in created}
        # nops were appended to the current block; rebuild every block,
        # dropping stray nops and splicing them before their target.
        for bb2 in nc.main_func.blocks:
            out = []
            for inst in bb2.instructions:
                if id(inst) in created_ids:
                    continue
                out.extend(inserts.get(id(inst), ()))
                out.append(inst)
            bb2.instructions[:] = out


def _classify_mask(maskT):
    """Tile class per (ik sk-chunk, jq sq-block) of maskT [sk, sq]."""
    cls = {}
    sk = np.arange(S)[:, None]
    sq = np.arange(S)[None, :]
    causal = np.where(sq >= sk, 0.0, -1e9).astype(np.float32)
    for ik in range(NKC):
        for jq in range(NSB):
            t = maskT[ik * P:(ik + 1) * P, jq * SBLK:(jq + 1) * SBLK]
            if np.all(t <= -1e8):
                cls[(ik, jq)] = "SKIP"
            elif np.all(t == 0.0):
                cls[(ik, jq)] = "FREE"
            elif np.array_equal(
                    t, causal[ik * P:(ik + 1) * P, jq * SBLK:(jq + 1) * SBLK]):
                cls[(ik, jq)] = "DIAG"
            else:
                cls[(ik, jq)] = "MIX"
    return cls


def _build(cls_key, cls, no_cc=False):
    import concourse.bass as bass
    import concourse.mybir as mybir
    import concourse.tile as tile
    from concourse import masks

    _install_drain_patch(tile, mybir)
    f16, f32 = mybir.dt.float16, mybir.dt.float32
    nc = bass.Bass("TRN2", target_bir_lowering=False, debug=False,
                   num_devices=NC)

    di = lambda n, s: nc.dram_tensor(n, s, f16, kind="ExternalInput").ap()
    df = lambda n, s: nc.dram_tensor(n, s, f32, kind="ExternalInput").ap()

    xt = di("xt", [NSB, P, DCH, SBLK])      # packed xT fp16 per s-block
    xc = df("xc", [D, SHARD])               # xT fp32, this core's shard cols
    cost = df("cost", [P, S])               # rope cos  [hd, s]
    s2t = df("s2t", [P, S])                 # rope sign-folded sin [hd, s]
    wq = di("wq", [P, DCH, 2 * P])
    wk = di("wk", [P, DCH, P])
    wv = di("wv", [P, DCH, P])
    wo = di("wo", [DCH, P, DCH, P])         # [dtile][p][echunk][m]
    wg = di("wg", [2, P, DCH, SBLK])        # [ffhalf][p][dchunk][ff]
    wu = di("wu", [2, P, DCH, SBLK])
    wd = di("wd", [DCH, P, FF // P, P])     # [dtile][p][ffchunk][m]
    maskt = df("maskt", [S, S])
    dmask = df("dmask", [4, P, SBLK])
    outt = nc.dram_tensor("outt", [D, SHARD], f32, kind="ExternalOutput").ap()

    with tile.TileContext(nc) as tc:
        with (
            tc.tile_pool(name="pers", bufs=1) as pers,
            tc.tile_pool(name="dram", bufs=1, space="DRAM") as dram,
        ):
            ones = pers.tile([P, P], f16, name="ones")
            nc.vector.memset(ones[:], 1.0)
            ident = pers.tile([P, P], f16, name="ident")
            masks.make_identity(nc, ident[:])
            permT = pers.tile([P, P], f16, name="permT")  # rot-half permute
            nc.gpsimd.memset(permT[:], 0.0)
            for base in (-64, 64):
                nc.gpsimd.affine_select(
                    out=permT[:], in_=permT[:],
                    compare_op=mybir.AluOpType.not_equal,
                    fill=1.0, base=base, pattern=[[-1, P]],
                    channel_multiplier=1)

            shift = pers.tile([P, 1], f32, name="shift")
            nc.vector.memset(shift[:], EXP_SHIFT)
            dm_sb = pers.tile([P, 4, SBLK], f32, name="dm_sb")
            nc.sync.dma_start(dm_sb[:], dmask[:].rearrange("k p s -> p k s"))
            x1t = pers.tile([P, DCH, SHARD], f32, name="x1t")
            attn_rb = pers.tile([P, 2, NC, SHARD], f16, name="attn_rb")
            wgu_sb = pers.tile([P, 2, DCH, SBLK], f16, name="wgu_sb")

            a2a1_in = [dram.tile([NC, P, SHARD], f16, name=f"a2a1i{_h}") for _h in range(2)]
            a2a1_out = [dram.tile([NC, P, SHARD], f16, name=f"a2a1o{_h}") for _h in range(2)]
            ag2_in = dram.tile([D, SHARD], f16)
            ag2_out = dram.tile([NC * D, SHARD], f16, addr_space="Shared")
            a2a3_in = [dram.tile([NC, FFSH // 2, SHARD], f16, name=f"a2a3i{_h}") for _h in range(2)]
            a2a3_out = [dram.tile([NC, FFSH // 2, SHARD], f16, name=f"a2a3o{_h}") for _h in range(2)]

            # ======== A: norm1 stats + QKV + RoPE + v transpose ========
            _pAB_cm = tc.tile_pool(name="pAB", bufs=1)
            pAB = _pAB_cm.__enter__()
            qh = [pAB.tile([P, S], f16, name=f"qh{i}") for i in range(2)]
            kh = pAB.tile([P, S], f16, name="kh")
            vnat = pAB.tile([P, NKC, P], f16, name="vnat")
            attn_st = pAB.tile([P, 2, S], f16, name="attn_st")
            with (
                tc.tile_pool(name="pA", bufs=2) as pA,
                tc.tile_pool(name="pAx", bufs=2) as pAx,
                tc.tile_pool(name="pAw", bufs=1) as pAw,
                tc.tile_pool(name="psA", bufs=2, space="PSUM") as psA,
                tc.tile_pool(name="psAv", bufs=2, space="PSUM") as psAv,
            ):
                wq_sb0 = pAw.tile([P, DCH, 2 * P], f16, name="wq_sb")
                nc.sync.dma_start(wq_sb0[:], wq[:])
                wk_sb0 = pAw.tile([P, DCH, P], f16, name="wk_sb")
                nc.sync.dma_start(wk_sb0[:], wk[:])
                wv_sb0 = pAw.tile([P, DCH, P], f16, name="wv_sb")
                nc.sync.dma_start(wv_sb0[:], wv[:])
                wq_sb = [wq_sb0[:, _i] for _i in range(DCH)]
                wk_sb = [wk_sb0[:, _i] for _i in range(DCH)]
                wv_sb = [wv_sb0[:, _i] for _i in range(DCH)]

                for j in range(NSB):
                    sl = slice(j * SBLK, (j + 1) * SBLK)
                    xt_h0 = pAx.tile([P, DCH // 2, SBLK], f16, name="xt_h0")
                    nc.sync.dma_start(xt_h0[:], xt[j, :, :DCH // 2])
                    xt_h1 = pAx.tile([P, DCH // 2, SBLK], f16, name="xt_h1")
                    nc.sync.dma_start(xt_h1[:], xt[j, :, DCH // 2:])
                    xt_sb = [xt_h0[:, _i] for _i in range(DCH // 2)] + \
                            [xt_h1[:, _i] for _i in range(DCH // 2)]
                    var_ps = psAv.tile([P, SBLK], f32, name="var")
                    for i in range(DCH):
                        xsq = pA.tile([P, SBLK], f16, name="xsq")
                        nc.vector.tensor_mul(xsq[:], xt_sb[i][:],
                                             xt_sb[i][:])
                        nc.tensor.matmul(var_ps[:], ones[:], xsq[:],
                                         start=(i == 0), stop=(i == DCH - 1))
                    t1 = pA.tile([P, SBLK], f32, name="t1")
                    nc.vector.tensor_scalar(
                        t1[:], var_ps[:], 1.0 / D, EPS,
                        mybir.AluOpType.mult, mybir.AluOpType.add)
                    t2 = pA.tile([P, SBLK], f32, name="t2")
                    nc.vector.reciprocal(t2[:], t1[:])
                    rstd = pA.tile([P, SBLK], f32, name="rstd")
                    nc.scalar.sqrt(rstd[:], t2[:])
                    cosj = pA.tile([P, SBLK], f32, name="cosj")
                    nc.sync.dma_start(cosj[:], cost[:, sl])
                    s2j = pA.tile([P, SBLK], f32, name="s2j")
                    nc.sync.dma_start(s2j[:], s2t[:, sl])
                    cr = pA.tile([P, SBLK], f32, name="cr")
                    nc.vector.tensor_mul(cr[:], cosj[:], rstd[:])
                    sr = pA.tile([P, SBLK], f32, name="sr")
                    nc.vector.tensor_mul(sr[:], s2j[:], rstd[:])

                    for (wsb, col0, dst) in (
                        (wq_sb, 0, qh[0]), (wq_sb, P, qh[1]), (wk_sb, 0, kh)
                    ):
                        ps = psA.tile([P, SBLK], f32, name="mm")
                        for i in range(DCH):
                            nc.tensor.matmul(
                                ps[:], wsb[i][:, col0:col0 + P],
                                xt_sb[i][:],
                                start=(i == 0), stop=(i == DCH - 1))
                        z16 = pA.tile([P, SBLK], f16, name="z16")
                        nc.vector.tensor_copy(z16[:], ps[:])
                        rps = psA.tile([P, SBLK], f32, name="rot")
                        nc.tensor.matmul(rps[:], permT[:], z16[:],
                                         start=True, stop=True)
                        av = pA.tile([P, SBLK], f32, name="av")
                        nc.vector.tensor_mul(av[:], ps[:], cr[:])
                        bv = pA.tile([P, SBLK], f32, name="bv")
                        nc.vector.tensor_mul(bv[:], rps[:], sr[:])
                        nc.vector.tensor_add(dst[:, sl], av[:], bv[:])

                    ps = psA.tile([P, SBLK], f32, name="mm")
                    for i in range(DCH):
                        nc.tensor.matmul(ps[:], wv_sb[i][:], xt_sb[i][:],
                                         start=(i == 0), stop=(i == DCH - 1))
                    vs = pA.tile([P, SBLK], f16, name="vs")
                    nc.vector.tensor_mul(vs[:], ps[:], rstd[:])
                    for t in range(SBLK // P):
                        tps = psAv.tile([P, P], f16, name="vt")
                        nc.tensor.transpose(tps[:], vs[:, t * P:(t + 1) * P],
                                            ident[:])
                        nc.vector.tensor_copy(
                            vnat[:, j * (SBLK // P) + t, :], tps[:])

            _pDh_cm = tc.tile_pool(name="pDh", bufs=2)
            pDh = _pDh_cm.__enter__()
            _pWd_cm = tc.tile_pool(name="pWd", bufs=2)
            pWd = _pWd_cm.__enter__()
            _pWo_cm = tc.tile_pool(name="pWo", bufs=3)
            pWo = _pWo_cm.__enter__()
            # ======== B: attention in scoresT layout ========
            with (
                tc.tile_pool(name="pB", bufs=4) as pB,
                tc.tile_pool(name="psB", bufs=4, space="PSUM") as psB,
                tc.tile_pool(name="psBa", bufs=2, space="PSUM") as psBa,
            ):
                for h in range(2):
                    for jq in range(NSB):
                        slq = slice(jq * SBLK, (jq + 1) * SBLK)
                        live = [ik for ik in range(NKC)
                                if cls[(ik, jq)] != "SKIP"]
                        live.sort(key=lambda ik: 0 if cls[(ik, jq)] in
                                  ("DIAG", "MIX") else 1)
                        dn_ps = psBa.tile([P, SBLK], f32, name="dn")
                        at_ps = psBa.tile([P, SBLK], f32, name="at")
                        for n, ik in enumerate(live):
                            c = cls[(ik, jq)]
                            sc = psB.tile([P, SBLK], f32, name="sc")
                            nc.tensor.matmul(
                                sc[:], kh[:, ik * P:(ik + 1) * P],
                                qh[h][:, slq], start=True, stop=True)
                            pr = pB.tile([P, SBLK], f16, name="pr")
                            if c == "FREE":
                                nc.scalar.activation(
                                    pr[:], sc[:],
                                    mybir.ActivationFunctionType.Exp,
                                    bias=shift[:, :])
                            elif c == "DIAG":
                                r = ik - 4 * jq
                                assert 0 <= r < 4, (ik, jq)
                                cp = pB.tile([P, SBLK], f32, name="cp")
                                nc.vector.tensor_add(cp[:], sc[:],
                                                     dm_sb[:, r, :])
                                nc.scalar.activation(
                                    pr[:], cp[:],
                                    mybir.ActivationFunctionType.Exp,
                                    bias=shift[:, :])
                            else:  # MIX
                                mk = pB.tile([P, SBLK], f32, name="mk")
                                nc.sync.dma_start(
                                    mk[:], maskt[ik * P:(ik + 1) * P, slq])
                                cp = pB.tile([P, SBLK], f32, name="cp")
                                nc.vector.tensor_add(cp[:], sc[:], mk[:])
                                nc.scalar.activation(
                                    pr[:], cp[:],
                                    mybir.ActivationFunctionType.Exp,
                                    bias=shift[:, :])
                            nc.tensor.matmul(dn_ps[:], ones[:], pr[:],
                                             start=(n == 0),
                                             stop=(n == len(live) - 1))
                            nc.tensor.matmul(at_ps[:], vnat[:, ik, :], pr[:],
                                             start=(n == 0),
                                             stop=(n == len(live) - 1))
                        rc = pB.tile([P, SBLK], f32, name="rc")
                        nc.vector.reciprocal(rc[:], dn_ps[:])
                        nc.vector.tensor_mul(attn_st[:, h, slq], at_ps[:],
                                             rc[:])
                    nc.sync.dma_start(
                        a2a1_in[h][:].rearrange("c p s -> p c s"),
                        attn_st[:, h].rearrange("p (c s) -> p c s", c=NC))
                    if not no_cc:
                        nc.gpsimd.collective_compute(
                            "AllToAll", mybir.AluOpType.bypass,
                            replica_groups=[list(range(NC))],
                            ins=[a2a1_in[h][:].opt()],
                            outs=[a2a1_out[h][:].opt()])

            # ======== C: o_proj (seq-shard) + residual + norm2 ========
            with (
                tc.tile_pool(name="pC", bufs=3) as pC,
                tc.tile_pool(name="pCr", bufs=1) as pCr,
                tc.tile_pool(name="psC", bufs=2, space="PSUM") as psC,
                tc.tile_pool(name="psCv", bufs=1, space="PSUM") as psCv,
            ):
                for h in range(2):
                    nc.sync.dma_start(
                        attn_rb[:, h],
                        a2a1_out[h][:].rearrange("c p s -> p c s"))
                var2 = psCv.tile([P, SHARD], f32, name="var2")
                for i in range(DCH):
                    wo_sb = pWo.tile([P, DCH, P], f16, name="wo_sb")
                    nc.sync.dma_start(wo_sb[:], wo[i])
                    ps = psC.tile([P, SHARD], f32, name="wops")
                    for h in range(2):
                        for cc in range(NC):
                            nc.tensor.matmul(
                                ps[:], wo_sb[:, 2 * cc + h, :],
                                attn_rb[:, h, cc, :],
                                start=(h == 0 and cc == 0),
                                stop=(h == 1 and cc == NC - 1))
                    xci = pC.tile([P, SHARD], f32, name="xci")
                    nc.sync.dma_start(
                        xci[:], xc[:].rearrange("(n p) s -> p n s", p=P)[:, i])
                    nc.vector.tensor_add(x1t[:, i, :], ps[:], xci[:])
                    sq2 = pC.tile([P, SHARD], f16, name="sq2")
                    nc.vector.tensor_mul(sq2[:], x1t[:, i, :], x1t[:, i, :])
                    nc.tensor.matmul(var2[:], ones[:], sq2[:],
                                     start=(i == 0), stop=(i == DCH - 1))
                u1 = pCr.tile([P, SHARD], f32, name="u1")
                nc.vector.tensor_scalar(
                    u1[:], var2[:], 1.0 / D, EPS,
                    mybir.AluOpType.mult, mybir.AluOpType.add)
                u2 = pCr.tile([P, SHARD], f32, name="u2")
                nc.vector.reciprocal(u2[:], u1[:])
                rstd2 = pCr.tile([P, SHARD], f32, name="rstd2")
                nc.scalar.sqrt(rstd2[:], u2[:])
                for i in range(DCH):
                    h2i = pC.tile([P, SHARD], f16, name="h2i")
                    nc.vector.tensor_mul(h2i[:], x1t[:, i, :], rstd2[:])
                    nc.sync.dma_start(
                        ag2_in[:].rearrange("(n p) s -> p n s", p=P)[:, i],
                        h2i[:])
            if not no_cc:
                nc.gpsimd.collective_compute(
                    "AllGather", mybir.AluOpType.bypass,
                    replica_groups=[list(range(NC))],
                    ins=[ag2_in[:].opt()], outs=[ag2_out[:].opt()])

            # ======== D: gate/up (ff-shard) + silu*up ========
            with (
                tc.tile_pool(name="pD", bufs=2) as pD,
                tc.tile_pool(name="psD", bufs=4, space="PSUM") as psD,
            ):
                for half in range(2):
                    wg_sb = wgu_sb[:, 0]
                    nc.sync.dma_start(wg_sb, wg[half])
                    wu_sb = wgu_sb[:, 1]
                    nc.sync.dma_start(wu_sb, wu[half])
                    for j in range(NSB):
                        h2rb = pDh.tile([P, DCH, 2, SHARD], f16, name="h2rb")
                        for cc in range(2):
                            nc.sync.dma_start(
                                h2rb[:, :, cc, :],
                                ag2_out[:].rearrange(
                                    "(c n p) s -> p n c s", c=NC, p=P
                                )[:, :, 2 * j + cc, :])
                        for ft in range(SBLK // P):
                            fsl = slice(ft * P, (ft + 1) * P)
                            psg = psD.tile([P, SBLK], f32, name="psg")
                            for i in range(DCH):
                                nc.tensor.matmul(
                                    psg[:], wg_sb[:, i, fsl], h2rb[:, i],
                                    start=(i == 0), stop=(i == DCH - 1))
                            psu = psD.tile([P, SBLK], f32, name="psu")
                            for i in range(DCH):
                                nc.tensor.matmul(
                                    psu[:], wu_sb[:, i, fsl], h2rb[:, i],
                                    start=(i == 0), stop=(i == DCH - 1))
                            slv = pD.tile([P, SBLK], f32, name="slv")
                            nc.scalar.activation(
                                slv[:], psg[:],
                                mybir.ActivationFunctionType.Silu)
                            gt = pD.tile([P, SBLK], f16, name="gt")
                            nc.vector.tensor_mul(gt[:], slv[:], psu[:])
                            nc.sync.dma_start(
                                a2a3_in[half][:].rearrange(
                                    "c (n p) s -> p n c s", p=P
                                )[:, ft, 2 * j:2 * j + 2, :],
                                gt[:].rearrange("p (c s) -> p c s", c=2))
                    if not no_cc:
                        nc.gpsimd.collective_compute(
                            "AllToAll", mybir.AluOpType.bypass,
                            replica_groups=[list(range(NC))],
                            ins=[a2a3_in[half][:].opt()],
                            outs=[a2a3_out[half][:].opt()])

            # ======== E: down_proj (seq-shard) + final residual ========
            with (
                tc.tile_pool(name="pE", bufs=2) as pE,
                tc.tile_pool(name="pEg", bufs=1) as pEg,
                tc.tile_pool(name="psE", bufs=2, space="PSUM") as psE,
            ):
                grb = [pEg.tile([P, FF // (2 * P), SHARD], f16,
                                name=f"grb{hf}") for hf in range(2)]
                for hf in range(2):
                    nc.sync.dma_start(
                        grb[hf][:],
                        a2a3_out[hf][:].rearrange("c (n p) s -> p (c n) s",
                                                  p=P))
                for i in range(DCH):
                    wd_sb = pWd.tile([P, FF // P, P], f16, name="wd_sb")
                    nc.sync.dma_start(wd_sb[:], wd[i])
                    ps = psE.tile([P, SHARD], f32, name="dps")
                    nmm = 0
                    for hf in range(2):
                        for cc in range(NC):
                            for n in range(4):
                                fg = cc * 8 + hf * 4 + n
                                nc.tensor.matmul(
                                    ps[:], wd_sb[:, fg, :],
                                    grb[hf][:, cc * 4 + n, :],
                                    start=(nmm == 0), stop=(nmm == 63))
                                nmm += 1
                    ot = pE.tile([P, SHARD], f32, name="ot")
                    nc.vector.tensor_add(ot[:], ps[:], x1t[:, i, :])
                    nc.sync.dma_start(
                        outt[:].rearrange("(n p) s -> p n s", p=P)[:, i], ot[:])
            _pWo_cm.__exit__(None, None, None)
            _pWd_cm.__exit__(None, None, None)
            _pDh_cm.__exit__(None, None, None)
            _pAB_cm.__exit__(None, None, None)
    import concourse.mybir as _mybir
    _split_waits(nc, _mybir)
    return nc


def _host_prep(inputs):
    x = np.ascontiguousarray(inputs["hidden_states"][0])          # [S, D]
    mask = np.ascontiguousarray(inputs["attention_mask"][0, 0])   # [sq, sk]
    maskT = np.ascontiguousarray(mask.T)                          # [sk, sq]
    ln1, ln2 = inputs["ln1_w"], inputs["ln2_w"]
    Wq, Wk, Wv, Wo = inputs["Wq"], inputs["Wk"], inputs["Wv"], inputs["Wo"]
    Wg, Wu, Wd = inputs["Wg"], inputs["Wu"], inputs["Wd"]

    xT = np.ascontiguousarray(x.T)                                # [D, S]
    xT16 = xT.astype(np.float16)
    # packed xt: [j sblk][p][n dchunk][m] = xT[n*128+p, j*512+m]
    xtp = np.ascontiguousarray(
        xT16.reshape(DCH, P, NSB, SBLK).transpose(2, 1, 0, 3))

    inv_freq = 1.0 / (THETA ** (np.arange(0, HD, 2, dtype=np.float32) / HD))
    t = np.arange(S, dtype=np.float32)
    freqs = np.outer(t, inv_freq)
    emb = np.concatenate([freqs, freqs], -1)                      # [S, HD]
    cosT = np.ascontiguousarray(np.cos(emb).T.astype(np.float32))  # [HD, S]
    sinT = np.sin(emb).T.astype(np.float32)
    s2T = sinT.copy()
    s2T[:64] = -s2T[:64]
    s2T = np.ascontiguousarray(s2T)

    scale = 1.0 / np.sqrt(HD)
    Wq_f = (ln1[:, None] * Wq * scale).astype(np.float16)   # [D, H*HD]
    Wk_f = (ln1[:, None] * Wk).astype(np.float16)
    Wv_f = (ln1[:, None] * Wv).astype(np.float16)
    Wg_f = (ln2[:, None] * Wg).astype(np.float16)
    Wu_f = (ln2[:, None] * Wu).astype(np.float16)
    Wo16 = Wo.astype(np.float16)                            # [H*HD, D]
    Wd16 = Wd.astype(np.float16)                            # [FF, D]

    # packed wo: [i dtile][p][e chunk][m] = Wo[e*128+p, i*128+m]
    wop = np.ascontiguousarray(
        Wo16.reshape(DCH, P, DCH, P).transpose(2, 1, 0, 3))
    # packed wd: [i][p][f chunk][m] = Wd[f*128+p, i*128+m]
    wdp = np.ascontiguousarray(
        Wd16.reshape(FF // P, P, DCH, P).transpose(2, 1, 0, 3))

    cls = _classify_mask(maskT)
    dmask = np.zeros((4, P, SBLK), np.float32)
    for (ik, jq), c in cls.items():
        if c == "DIAG":
            r = ik - 4 * jq
            assert 0 <= r < 4, "DIAG tile off the ik==4*jq+r band"
            dmask[r] = maskT[ik * P:(ik + 1) * P, jq * SBLK:(jq + 1) * SBLK]
    in_maps = []
    for c in range(NC):
        qsl = slice(2 * P * c, 2 * P * (c + 1))
        kvsl = slice(P * (c // 2), P * (c // 2) + P)
        ffsl = slice(FFSH * c, FFSH * (c + 1))
        ssl = slice(SHARD * c, SHARD * (c + 1))
        wq_c = Wq_f[:, qsl]    # [D, 256]
        wk_c = Wk_f[:, kvsl]   # [D, 128]
        wv_c = Wv_f[:, kvsl]
        wg_c = Wg_f[:, ffsl]   # [D, 1024]
        wu_c = Wu_f[:, ffsl]
        in_maps.append({
            "xt": xtp,
            "xc": np.ascontiguousarray(xT[:, ssl]),
            "cost": cosT,
            "s2t": s2T,
            # [p][n dchunk][cols]
            "wq": np.ascontiguousarray(
                wq_c.reshape(DCH, P, 2 * P).transpose(1, 0, 2)),
            "wk": np.ascontiguousarray(
                wk_c.reshape(DCH, P, P).transpose(1, 0, 2)),
            "wv": np.ascontiguousarray(
                wv_c.reshape(DCH, P, P).transpose(1, 0, 2)),
            "wo": wop,
            # [half][p][n dchunk][ff 512] = Wg_f[n*128+p, half*512+m]
            "wg": np.ascontiguousarray(
                wg_c.reshape(DCH, P, 2, SBLK).transpose(2, 1, 0, 3)),
            "wu": np.ascontiguousarray(
                wu_c.reshape(DCH, P, 2, SBLK).transpose(2, 1, 0, 3)),
            "wd": wdp,
            "maskt": maskT,
            "dmask": dmask,
        })
    return in_maps, cls


def kernel(**inputs):
    from concourse import bass_utils

    in_maps, cls = _host_prep(inputs)
    cls_key = tuple(sorted(cls.items()))
    if cls_key not in _CACHE:
        _CACHE[cls_key] = _build(cls_key, cls)
    nc = _CACHE[cls_key]

    res = bass_utils.run_bass_kernel_spmd(
        nc, in_maps, core_ids=list(range(NC)))
    out = np.empty((S, D), dtype=np.float32)
    for c in range(NC):
        out[SHARD * c:SHARD * (c + 1), :] = res.results[c]["outt"].T
    return out[None]



# revision 33
# speedup vs baseline: 1.0270x; 1.0270x over previous
"""Trainium2 Bass kernel: Deepseek-style decoder layer (dense transformer),
tensor-parallel over 8 NeuronCores.

Per core: 2 Q heads + their GQA KV head; attention computed in transposed
(scoresT) layout. RMSNorm variance and softmax denominators are accumulated
elementwise on DVE (single ones-matmul each) instead of per-chunk PE
ones-matmuls. Phase D keeps the AllGathered h2 resident in SBUF across both
ff-halves and streams gate/up weights in 128-col chunks; phase E runs a
two-pass (hf0 first) down_proj with 1MB weight chunks prefetched on the Pool
DMA queue during phase D. All matmuls fp16 (full PE rate), fp32 PSUM
accumulation + fp32 residuals.
"""
import sys
import os
import numpy as np

for _p in ("/opt/trn_rl_repo", "/root/.axon_site/_ro/trn_rl_repo"):
    if os.path.isdir(_p) and _p not in sys.path:
        sys.path.append(_p)

B, S, D = 1, 2048, 2048
H, KVH, HD = 16, 4, 128
FF = 8192
EPS = 1e-6
THETA = 10000.0
NC = 8
P = 128
SBLK = 512
NSB = S // SBLK          # 4 s-blocks
DCH = D // P             # 16 d-chunks
SHARD = S // NC          # 256 tokens per core
FFSH = FF // NC          # 1024
NKC = S // P             # 16 sk-chunks
EXP_SHIFT = -8.0         # exp(x+shift): cancels in softmax ratio, keeps fp16
                         # probs far from overflow without a max pass

_CACHE = {}


def _install_drain_patch(tile_mod, mybir):
    """Stock tail-drain puts one wait per outstanding proc on a single sync-
    queue CTRL op, which supports only ONE wait -> chain single-wait drains."""
    from concourse.vector_clock import ScopedClock

    def _split(self, tick_clock, wait_clock):
        nc = self.nc
        drain_inst = nc.sync.drain()
        wait_clock.add_sem_waits(
            drain_inst.ins, ScopedClock({None: tick_clock.global_clock}))
        si = drain_inst.ins.sync_info
        if si is not None and len(si.on_wait) > 1:
            waits, upd = list(si.on_wait), list(si.on_update)
            drain_inst.ins.sync_info = mybir.SyncInfo(
                on_wait=waits[:1], on_update=[])
            rest = waits[1:]
            while rest:
                chunk, rest = rest[:1], rest[1:]
                extra = nc.sync.drain()
                extra.ins.sync_info = mybir.SyncInfo(
                    on_wait=chunk, on_update=([] if rest else upd))
        nc.all_engine_barrier()
        assert self.sems is not None
        popped = nc._tile_sem_poison_stack.pop()
        assert popped is self._sem_poison
        nc.clear_and_free_semaphores(list(self.sems.allocated().values()))
        nc.all_engine_barrier()

    tile_mod.TileContext._drain_and_barrier = _split


def _split_waits(nc, mybir):
    """HW allows one sync-wait per instruction on these queues; Tile can emit
    several at dependency joins. Insert same-engine no-ops, each carrying one
    excess wait, immediately before the offending instruction."""
    eng_map = {
        mybir.EngineType.DVE: nc.vector,
        mybir.EngineType.Activation: nc.scalar,
        mybir.EngineType.PE: nc.tensor,
        mybir.EngineType.Pool: nc.gpsimd,
        mybir.EngineType.SP: nc.sync,
    }
    for bb in nc.main_func.blocks:
        todo = []
        for inst in bb.instructions:
            si = getattr(inst, "sync_info", None)
            if si is not None and len(si.on_wait) > 1:
                todo.append(inst)
        if not todo:
            continue
        inserts = {}
        created = []
        for inst in todo:
            si = inst.sync_info
            waits = list(si.on_wait)
            nops = []
            for w in waits[:-1]:
                nop = eng_map[inst.engine].nop().ins
                nop.sync_info = mybir.SyncInfo(on_wait=[w], on_update=[])
                nops.append(nop)
                created.append(nop)
            inst.sync_info = mybir.SyncInfo(
                on_wait=[waits[-1]], on_update=list(si.on_update))
            inserts[id(inst)] = nops
        created_ids = {id(n) for n in created}
        # nops were appended to the current block; rebuild every block,
        # dropping stray nops and splicing them before their target.
        for bb2 in nc.main_func.blocks:
            out = []
            for inst in bb2.instructions:
                if id(inst) in created_ids:
                    continue
                out.extend(inserts.get(id(inst), ()))
                out.append(inst)
            bb2.instructions[:] = out


def _classify_mask(maskT):
    """Tile class per (ik sk-chunk, jq sq-block) of maskT [sk, sq]."""
    cls = {}
    sk = np.arange(S)[:, None]
    sq = np.arange(S)[None, :]
    causal = np.where(sq >= sk, 0.0, -1e9).astype(np.float32)
    for ik in range(NKC):
        for jq in range(NSB):
            t = maskT[ik * P:(ik + 1) * P, jq * SBLK:(jq + 1) * SBLK]
            if np.all(t <= -1e8):
                cls[(ik, jq)] = "SKIP"
            elif np.all(t == 0.0):
                cls[(ik, jq)] = "FREE"
            elif np.array_equal(
                    t, causal[ik * P:(ik + 1) * P, jq * SBLK:(jq + 1) * SBLK]):
                cls[(ik, jq)] = "DIAG"
            else:
                cls[(ik, jq)] = "MIX"
    return cls


def _build(cls_key, cls, no_cc=False):
    import concourse.bass as bass
    import concourse.mybir as mybir
    import concourse.tile as tile
    from concourse import masks

    _install_drain_patch(tile, mybir)
    f16, f32 = mybir.dt.float16, mybir.dt.float32
    nc = bass.Bass("TRN2", target_bir_lowering=False, debug=False,
                   num_devices=NC)

    di = lambda n, s: nc.dram_tensor(n, s, f16, kind="ExternalInput").ap()
    df = lambda n, s: nc.dram_tensor(n, s, f32, kind="ExternalInput").ap()

    xt = di("xt", [NSB, P, DCH, SBLK])      # packed xT fp16 per s-block
    xc = df("xc", [D, SHARD])               # xT fp32, this core's shard cols
    cost = di("cost", [P, S])               # rope cos  [hd, s] fp16
    s2t = di("s2t", [P, S])                 # rope sign-folded sin [hd, s] fp16
    wq = di("wq", [P, DCH, 2 * P])
    wk = di("wk", [P, DCH, P])
    wv = di("wv", [P, DCH, P])
    wo = di("wo", [DCH, P, DCH, P])         # [dtile][p][echunk][m]
    wg = di("wg", [2, P, DCH, SBLK])        # [ffhalf][p][dchunk][ff]
    wu = di("wu", [2, P, DCH, SBLK])
    wd = di("wd", [DCH, P, FF // P, P])     # [dtile][p][ffchunk][m]
    maskt = df("maskt", [S, S])
    dmask = df("dmask", [4, P, SBLK])
    outt = nc.dram_tensor("outt", [D, SHARD], f32, kind="ExternalOutput").ap()

    with tile.TileContext(nc) as tc:
        with (
            tc.tile_pool(name="pers", bufs=1) as pers,
            tc.tile_pool(name="dram", bufs=1, space="DRAM") as dram,
        ):
            ones = pers.tile([P, P], f16, name="ones")
            nc.vector.memset(ones[:], 1.0)
            ident = pers.tile([P, P], f16, name="ident")
            masks.make_identity(nc, ident[:])
            permT = pers.tile([P, P], f16, name="permT")  # rot-half permute
            nc.gpsimd.memset(permT[:], 0.0)
            for base in (-64, 64):
                nc.gpsimd.affine_select(
                    out=permT[:], in_=permT[:],
                    compare_op=mybir.AluOpType.not_equal,
                    fill=1.0, base=base, pattern=[[-1, P]],
                    channel_multiplier=1)

            shift = pers.tile([P, 1], f32, name="shift")
            nc.vector.memset(shift[:], EXP_SHIFT)
            x1t = pers.tile([P, DCH, SHARD], f32, name="x1t")

            a2a1_in = [dram.tile([NC, P, SHARD], f16, name=f"a2a1i{_h}") for _h in range(2)]
            a2a1_out = [dram.tile([NC, P, SHARD], f16, name=f"a2a1o{_h}") for _h in range(2)]
            ag2_in = dram.tile([D, SHARD], f16)
            ag2_out = dram.tile([NC * D, SHARD], f16, addr_space="Shared")
            a2a3_in = [dram.tile([NC, FFSH // 2, SHARD], f16, name=f"a2a3i{_h}") for _h in range(2)]
            a2a3_out = [dram.tile([NC, FFSH // 2, SHARD], f16, name=f"a2a3o{_h}") for _h in range(2)]

            # C-epilogue tiles live in a pool that stays open through D/E so
            # the D pools never reuse their SBUF: otherwise D's first DMAs
            # inherit WAR waits on the h2 staging reads and the PE drains.
            _pCe_cm = tc.tile_pool(name="pCe", bufs=1)
            pCe = _pCe_cm.__enter__()
            h2stage = pCe.tile([P, DCH, SHARD], f16, name="h2stage")
            sqacc = pCe.tile([P, SHARD], f16, name="sqacc")
            u1 = pCe.tile([P, SHARD], f32, name="u1")
            u2 = pCe.tile([P, SHARD], f32, name="u2")
            rstd2 = pCe.tile([P, SHARD], f32, name="rstd2")

            # long-lived pools: pAB (A+B tensors), pRB (attn_rb + diag masks,
            # lives B..C), pWo (o_proj weights, lives B..C).  All three close
            # after phase C to free SBUF for phase D's resident h2.
            # gate/up weight-chunk pool lives below the A-C pools so its
            # prefetches (issued at C start) carry no WAR waits.
            _pDw_cm = tc.tile_pool(name="pDw", bufs=3)
            pDw = _pDw_cm.__enter__()
            wgu_tiles = {}

            def _issue_wgu(half, ft, eng):
                fsl = slice(ft * P, (ft + 1) * P)
                wgc = pDw.tile([P, DCH, P], f16, name="wgc")
                eng.dma_start(wgc[:], wg[half, :, :, fsl])
                wuc = pDw.tile([P, DCH, P], f16, name="wuc")
                eng.dma_start(wuc[:], wu[half, :, :, fsl])
                wgu_tiles[(half, ft)] = (wgc, wuc)

            _pAB_cm = tc.tile_pool(name="pAB", bufs=1)
            pAB = _pAB_cm.__enter__()
            qh = [pAB.tile([P, S], f16, name=f"qh{i}") for i in range(2)]
            kh = pAB.tile([P, S], f16, name="kh")
            vnat = pAB.tile([P, NKC, P], f16, name="vnat")
            attn_st = pAB.tile([P, 2, S], f16, name="attn_st")
            _pRB_cm = tc.tile_pool(name="pRB", bufs=1)
            pRB = _pRB_cm.__enter__()
            attn_rb = [pRB.tile([P, NC, SHARD], f16, name=f"attn_rb{_h}")
                       for _h in range(2)]
            dm_sb = pRB.tile([P, 4, SBLK], f32, name="dm_sb")
            _pWo_cm = tc.tile_pool(name="pWo", bufs=3)
            pWo = _pWo_cm.__enter__()

            # ======== A: norm1 stats + QKV + RoPE + v transpose ========
            with (
                tc.tile_pool(name="pA", bufs=2) as pA,
                tc.tile_pool(name="pAx", bufs=2) as pAx,
                tc.tile_pool(name="pAw", bufs=1) as pAw,
                tc.tile_pool(name="psA", bufs=2, space="PSUM") as psA,
                tc.tile_pool(name="psAv", bufs=2, space="PSUM") as psAv,
            ):
                QTR = DCH // 4

                def load_xt(j, interleave=()):
                    """4 separate quarter tiles per s-block: per-tile dep
                    tracking lets the first matmuls start after 0.7us of DMA
                    instead of waiting a full-block transfer."""
                    out = []
                    inter = list(interleave)
                    for q in range(4):
                        t = pAx.tile([P, QTR, SBLK], f16, name=f"xt_q{q}")
                        nc.sync.dma_start(
                            t[:], xt[j, :, q * QTR:(q + 1) * QTR])
                        out += [t[:, _i] for _i in range(QTR)]
                        if inter:
                            inter.pop(0)()
                    return out

                # DMA order matters: x-block 0 quarters and wq halves are
                # interleaved at the head of the queue so the first matmul
                # starts ~2us in, not ~15us.
                xts = {}
                wq_a = pAw.tile([P, DCH // 2, 2 * P], f16, name="wq_a")
                wq_b = pAw.tile([P, DCH // 2, 2 * P], f16, name="wq_b")
                wk_sb0 = pAw.tile([P, DCH, P], f16, name="wk_sb")
                wv_sb0 = pAw.tile([P, DCH, P], f16, name="wv_sb")
                cos0 = pA.tile([P, SBLK], f16, name="cosj")
                s20 = pA.tile([P, SBLK], f16, name="s2j")
                xts[0] = load_xt(0, interleave=(
                    lambda: nc.scalar.dma_start(wq_a[:], wq[:, :DCH // 2]),
                    lambda: (nc.gpsimd.dma_start(cos0[:], cost[:, 0:SBLK]),
                             nc.gpsimd.dma_start(s20[:], s2t[:, 0:SBLK])),
                    lambda: nc.scalar.dma_start(wq_b[:], wq[:, DCH // 2:]),
                    lambda: (nc.scalar.dma_start(wk_sb0[:], wk[:]),
                             nc.scalar.dma_start(wv_sb0[:], wv[:]),
                             nc.gpsimd.dma_start(
                                 dm_sb[:],
                                 dmask[:].rearrange("k p s -> p k s"))),
                ))
                wq_sb = [wq_a[:, _i] for _i in range(DCH // 2)] + \
                        [wq_b[:, _i] for _i in range(DCH // 2)]
                wk_sb = [wk_sb0[:, _i] for _i in range(DCH)]
                wv_sb = [wv_sb0[:, _i] for _i in range(DCH)]

                for j in range(NSB):
                    sl = slice(j * SBLK, (j + 1) * SBLK)
                    if j == 0:
                        cosj, s2j = cos0, s20
                    else:
                        cosj = pA.tile([P, SBLK], f16, name="cosj")
                        nc.sync.dma_start(cosj[:], cost[:, sl])
                        s2j = pA.tile([P, SBLK], f16, name="s2j")
                        nc.sync.dma_start(s2j[:], s2t[:, sl])
                    xt_sb = xts.pop(j)
                    if j + 1 < NSB:
                        xts[j + 1] = load_xt(j + 1)

                    # x^2 accumulated elementwise on DVE (fp16, 4 parallel
                    # sub-chains to keep the serial latency short); the ops
                    # are interleaved between the projection epilogues so the
                    # rope z16 copies (which gate the PE rot matmuls) never
                    # queue behind the accumulation chain on DVE.
                    vsub = [pA.tile([P, SBLK], f16, name=f"vacc{_q}")
                            for _q in range(4)]
                    vacc = vsub[0]
                    vacc_ops = []
                    for i in range(DCH):
                        def _sq(i=i):
                            q, r = i % 4, i // 4
                            if r == 0:
                                nc.vector.tensor_mul(
                                    vsub[q][:], xt_sb[i][:], xt_sb[i][:])
                            else:
                                xsq = pA.tile([P, SBLK], f16, name="xsq")
                                nc.vector.tensor_mul(xsq[:], xt_sb[i][:],
                                                     xt_sb[i][:])
                                nc.vector.tensor_add(vsub[q][:], vsub[q][:],
                                                     xsq[:])
                        vacc_ops.append(_sq)

                    def _vmerge():
                        nc.vector.tensor_add(vsub[0][:], vsub[0][:],
                                             vsub[1][:])
                        nc.vector.tensor_add(vsub[2][:], vsub[2][:],
                                             vsub[3][:])
                        nc.vector.tensor_add(vsub[0][:], vsub[0][:],
                                             vsub[2][:])
                    vacc_ops.append(_vmerge)

                    def run_vacc(k):
                        while vacc_ops and k > 0:
                            vacc_ops.pop(0)()
                            k -= 1

                    # projections: rstd is applied in the deferred epilogue,
                    # so av/bv depend only on the rope tables and free their
                    # PSUM slots immediately.
                    sms = []
                    for (wsb, col0, dst) in (
                        (wq_sb, 0, qh[0]), (wq_sb, P, qh[1]), (wk_sb, 0, kh)
                    ):
                        ps = psA.tile([P, SBLK], f32, name="mm")
                        for i in range(DCH):
                            nc.tensor.matmul(
                                ps[:], wsb[i][:, col0:col0 + P],
                                xt_sb[i][:],
                                start=(i == 0), stop=(i == DCH - 1))
                        z16 = pA.tile([P, SBLK], f16, name="z16")
                        nc.vector.tensor_copy(z16[:], ps[:])
                        rps = psA.tile([P, SBLK], f32, name="rot")
                        nc.tensor.matmul(rps[:], permT[:], z16[:],
                                         start=True, stop=True)
                        # all-f16 SBUF epilogue ops hit the DVE 4x mode
                        av = pA.tile([P, SBLK], f16, name="av")
                        nc.vector.tensor_mul(av[:], z16[:], cosj[:])
                        bv = pA.tile([P, SBLK], f16, name="bv")
                        nc.vector.tensor_mul(bv[:], rps[:], s2j[:])
                        sm = pA.tile([P, SBLK], f16, name=f"sm{len(sms)}")
                        nc.vector.tensor_add(sm[:], av[:], bv[:])
                        sms.append((sm, dst))
                        run_vacc(12)
                    run_vacc(DCH + 1)

                    var_ps = psAv.tile([P, SBLK], f32, name="var")
                    nc.tensor.matmul(var_ps[:], ones[:], vacc[:],
                                     start=True, stop=True)
                    t1 = pA.tile([P, SBLK], f32, name="t1")
                    nc.vector.tensor_scalar(
                        t1[:], var_ps[:], 1.0 / D, EPS,
                        mybir.AluOpType.mult, mybir.AluOpType.add)
                    t2 = pA.tile([P, SBLK], f32, name="t2")
                    nc.vector.reciprocal(t2[:], t1[:])
                    rstd = pA.tile([P, SBLK], f16, name="rstd")
                    nc.scalar.sqrt(rstd[:], t2[:])
                    for sm, dst in sms:
                        nc.vector.tensor_mul(dst[:, sl], sm[:], rstd[:])

                    ps = psA.tile([P, SBLK], f32, name="mm")
                    for i in range(DCH):
                        nc.tensor.matmul(ps[:], wv_sb[i][:], xt_sb[i][:],
                                         start=(i == 0), stop=(i == DCH - 1))
                    vs = pA.tile([P, SBLK], f16, name="vs")
                    nc.vector.tensor_mul(vs[:], ps[:], rstd[:])
                    for t in range(SBLK // P):
                        tps = psAv.tile([P, P], f16, name="vt")
                        nc.tensor.transpose(tps[:], vs[:, t * P:(t + 1) * P],
                                            ident[:])
                        nc.vector.tensor_copy(
                            vnat[:, j * (SBLK // P) + t, :], tps[:])

            # ======== B: attention in scoresT layout ========
            with (
                tc.tile_pool(name="pB", bufs=4) as pB,
                tc.tile_pool(name="psB", bufs=4, space="PSUM") as psB,
                tc.tile_pool(name="psBa", bufs=2, space="PSUM") as psBa,
                tc.tile_pool(name="psBd", bufs=2, space="PSUM") as psBd,
            ):
                # prefetch the first o_proj weight tiles on the Pool queue
                wo_tiles = {}
                for i in range(3):
                    wo_sb = pWo.tile([P, DCH, P], f16, name="wo_sb")
                    (nc.gpsimd if i % 2 else nc.scalar).dma_start(
                        wo_sb[:], wo[i])
                    wo_tiles[i] = wo_sb
                # One flat pipeline across all (h, jq) groups: av matmuls
                # trail the scores stream by 2 tiles and each group's tail
                # (last avs + denominator + normalize) is emitted under the
                # next group's scores, so the PE never drains on the exp
                # chain latency.
                pend = []          # (at_ps, n, ik, pr, nlive)
                tail = []          # deferred per-group finish closures

                def flush_at(nmax):
                    while len(pend) > nmax:
                        atp, n0, ik0, pr0, nl = pend.pop(0)
                        nc.tensor.matmul(
                            atp[:], vnat[:, ik0, :], pr0[:],
                            start=(n0 == 0), stop=(n0 == nl - 1))
                        if n0 == nl - 1:
                            tail.pop(0)()

                for h in range(2):
                    for jq in range(NSB):
                        slq = slice(jq * SBLK, (jq + 1) * SBLK)
                        live = [ik for ik in range(NKC)
                                if cls[(ik, jq)] != "SKIP"]
                        # FREE tiles first: their exp reads the scores PSUM
                        # directly (no DVE mask-add), so the Act pipeline
                        # starts without waiting on the DVE backlog.
                        live.sort(key=lambda ik: 1 if cls[(ik, jq)] in
                                  ("DIAG", "MIX") else 0)
                        at_ps = psBa.tile([P, SBLK], f32, name="at")
                        pracc = pB.tile([P, SBLK], f16, name="pracc")

                        def finish(at_ps=at_ps, pracc=pracc, h=h, slq=slq):
                            dn_ps = psBd.tile([P, SBLK], f32, name="dn")
                            nc.tensor.matmul(dn_ps[:], ones[:], pracc[:],
                                             start=True, stop=True)
                            rc = pB.tile([P, SBLK], f32, name="rc")
                            nc.vector.reciprocal(rc[:], dn_ps[:])
                            nc.vector.tensor_mul(attn_st[:, h, slq],
                                                 at_ps[:], rc[:])
                        tail.append(finish)

                        for n, ik in enumerate(live):
                            c = cls[(ik, jq)]
                            sc = psB.tile([P, SBLK], f32, name="sc")
                            nc.tensor.matmul(
                                sc[:], kh[:, ik * P:(ik + 1) * P],
                                qh[h][:, slq], start=True, stop=True)
                            pr = pB.tile([P, SBLK], f16, name="pr")
                            if c == "FREE":
                                nc.scalar.activation(
                                    pr[:], sc[:],
                                    mybir.ActivationFunctionType.Exp,
                                    bias=shift[:, :])
                            elif c == "DIAG":
                                r = ik - 4 * jq
                                assert 0 <= r < 4, (ik, jq)
                                cp = pB.tile([P, SBLK], f32, name="cp")
                                nc.vector.tensor_add(cp[:], sc[:],
                                                     dm_sb[:, r, :])
                                nc.scalar.activation(
                                    pr[:], cp[:],
                                    mybir.ActivationFunctionType.Exp,
                                    bias=shift[:, :])
                            else:  # MIX
                                mk = pB.tile([P, SBLK], f32, name="mk")
                                nc.sync.dma_start(
                                    mk[:], maskt[ik * P:(ik + 1) * P, slq])
                                cp = pB.tile([P, SBLK], f32, name="cp")
                                nc.vector.tensor_add(cp[:], sc[:], mk[:])
                                nc.scalar.activation(
                                    pr[:], cp[:],
                                    mybir.ActivationFunctionType.Exp,
                                    bias=shift[:, :])
                            if n == 0:
                                nc.vector.tensor_copy(pracc[:], pr[:])
                            else:
                                nc.vector.tensor_add(pracc[:], pracc[:],
                                                     pr[:])
                            pend.append((at_ps, n, ik, pr, len(live)))
                            flush_at(2)
                    if h == 0:
                        flush_at(0)
                    staged = attn_st[:, h]

                    def stage_h(h=h, staged=staged):
                        nc.sync.dma_start(
                            a2a1_in[h][:].rearrange("c p s -> p c s"),
                            staged.rearrange("p (c s) -> p c s", c=NC))
                        if not no_cc:
                            nc.gpsimd.collective_compute(
                                "AllToAll", mybir.AluOpType.bypass,
                                replica_groups=[list(range(NC))],
                                ins=[a2a1_in[h][:].opt()],
                                outs=[a2a1_out[h][:].opt()])
                        for cc in range(NC):
                            (nc.gpsimd if cc % 2 else nc.scalar).dma_start(
                                attn_rb[h][:, cc], a2a1_out[h][cc])
                    if h == 0:
                        stage_h()
                    else:
                        flush_at(0)
                        stage_h()

            # ======== C: o_proj (seq-shard) + residual + norm2 ========
            with (
                tc.tile_pool(name="pC", bufs=3) as pC,
                tc.tile_pool(name="psC", bufs=2, space="PSUM") as psC,
                tc.tile_pool(name="psCv", bufs=1, space="PSUM") as psCv,
            ):
                _issue_wgu(0, 0, nc.sync)
                _issue_wgu(0, 1, nc.sync)
                for i in range(DCH):
                    if i in wo_tiles:
                        wo_sb = wo_tiles.pop(i)
                    else:
                        wo_sb = pWo.tile([P, DCH, P], f16, name="wo_sb")
                        (nc.gpsimd if i % 2 else nc.scalar).dma_start(
                            wo_sb[:], wo[i])
                    xci = pC.tile([P, SHARD], f32, name="xci")
                    nc.sync.dma_start(
                        xci[:], xc[:].rearrange("(n p) s -> p n s", p=P)[:, i])
                    ps = psC.tile([P, SHARD], f32, name="wops")
                    for h in range(2):
                        for cc in range(NC):
                            nc.tensor.matmul(
                                ps[:], wo_sb[:, 2 * cc + h, :],
                                attn_rb[h][:, cc, :],
                                start=(h == 0 and cc == 0),
                                stop=(h == 1 and cc == NC - 1))
                    nc.vector.tensor_add(x1t[:, i, :], ps[:], xci[:])
                    if i == 0:
                        nc.vector.tensor_mul(sqacc[:], x1t[:, i, :],
                                             x1t[:, i, :])
                    else:
                        sq2 = pC.tile([P, SHARD], f16, name="sq2")
                        nc.vector.tensor_mul(sq2[:], x1t[:, i, :],
                                             x1t[:, i, :])
                        nc.vector.tensor_add(sqacc[:], sqacc[:], sq2[:])
                var2 = psCv.tile([P, SHARD], f32, name="var2")
                nc.tensor.matmul(var2[:], ones[:], sqacc[:],
                                 start=True, stop=True)
                nc.vector.tensor_scalar(
                    u1[:], var2[:], 1.0 / D, EPS,
                    mybir.AluOpType.mult, mybir.AluOpType.add)
                nc.vector.reciprocal(u2[:], u1[:])
                nc.scalar.sqrt(rstd2[:], u2[:])
                # normalize into one staging tile, ship in two big DMAs so
                # the post-rstd2 tail is short for the AllGather.  Muls split
                # across DVE and Pool to halve the serial tail.
                for i in range(DCH):
                    nc.vector.tensor_mul(h2stage[:, i, :], x1t[:, i, :],
                                         rstd2[:])
                    if i in (DCH // 2 - 1, DCH - 1):
                        lo = 0 if i < DCH // 2 else DCH // 2
                        nc.scalar.dma_start(
                            ag2_in[:].rearrange("(n p) s -> p n s",
                                                p=P)[:, lo:i + 1],
                            h2stage[:, lo:i + 1, :])
            if not no_cc:
                nc.gpsimd.collective_compute(
                    "AllGather", mybir.AluOpType.bypass,
                    replica_groups=[list(range(NC))],
                    ins=[ag2_in[:].opt()], outs=[ag2_out[:].opt()])

            # A/B/C-lifetime pools close here: frees SBUF for phase D's
            # resident h2 + weight streams.
            _pWo_cm.__exit__(None, None, None)
            _pRB_cm.__exit__(None, None, None)
            _pAB_cm.__exit__(None, None, None)

            _pWd_cm = tc.tile_pool(name="pWd", bufs=3)
            pWd = _pWd_cm.__enter__()
            _pEg_cm = tc.tile_pool(name="pEg", bufs=1)
            pEg = _pEg_cm.__enter__()
            grb = [pEg.tile([P, FF // (2 * P), SHARD], f16, name=f"grb{hf}")
                   for hf in range(2)]
            # down_proj weight chunks, consumed in E's two-pass order
            wd_order = [(0, d) for d in range(8)] + \
                       [(1, d) for d in range(8)] + \
                       [hd for d in range(8, 16) for hd in ((0, d), (1, d))]
            wd_tiles = {}

            def _issue_wd(idx, eng):
                hf, dtl = wd_order[idx]
                t = pWd.tile([P, NC, 4, P], f16, name="wdc")
                eng.dma_start(
                    t[:],
                    wd[dtl].rearrange("p (c two n) m -> p c two n m",
                                      c=NC, two=2)[:, :, hf])
                wd_tiles[idx] = t

            # ======== D: gate/up (ff-shard) + silu*up ========
            with (
                tc.tile_pool(name="pDh2", bufs=1) as pDh2,
                tc.tile_pool(name="pD", bufs=2) as pD,
                tc.tile_pool(name="psD", bufs=4, space="PSUM") as psD,
            ):
                h2t = [pDh2.tile([P, DCH, 2, SHARD], f16, name=f"h2t{jj}")
                       for jj in range(NSB)]
                for jj in range(NSB):
                    for cc in range(2):
                        # alternate queues so h2t[0] lands in one transfer
                        # time, not two serialized ones
                        (nc.gpsimd if cc else nc.scalar).dma_start(
                            h2t[jj][:, :, cc, :],
                            ag2_out[:].rearrange("(c n p) s -> p n c s",
                                                 c=NC, p=P)[:, :, 2 * jj + cc, :])
                for half in range(2):
                    for ft in range(NSB):
                        if (half, ft) in wgu_tiles:
                            wgc, wuc = wgu_tiles.pop((half, ft))
                        else:
                            _issue_wgu(half, ft, nc.sync)
                            wgc, wuc = wgu_tiles.pop((half, ft))
                        for j in range(NSB):
                            psg = psD.tile([P, SBLK], f32, name="psg")
                            for i in range(DCH):
                                nc.tensor.matmul(
                                    psg[:], wgc[:, i], h2t[j][:, i],
                                    start=(i == 0), stop=(i == DCH - 1))
                            psu = psD.tile([P, SBLK], f32, name="psu")
                            for i in range(DCH):
                                nc.tensor.matmul(
                                    psu[:], wuc[:, i], h2t[j][:, i],
                                    start=(i == 0), stop=(i == DCH - 1))
                            slv = pD.tile([P, SBLK], f32, name="slv")
                            nc.scalar.activation(
                                slv[:], psg[:],
                                mybir.ActivationFunctionType.Silu)
                            gt = pD.tile([P, SBLK], f16, name="gt")
                            nc.vector.tensor_mul(gt[:], slv[:], psu[:])
                            nc.sync.dma_start(
                                a2a3_in[half][:].rearrange(
                                    "c (n p) s -> p n c s", p=P
                                )[:, ft, 2 * j:2 * j + 2, :],
                                gt[:].rearrange("p (c s) -> p c s", c=2))
                    if not no_cc:
                        nc.gpsimd.collective_compute(
                            "AllToAll", mybir.AluOpType.bypass,
                            replica_groups=[list(range(NC))],
                            ins=[a2a3_in[half][:].opt()],
                            outs=[a2a3_out[half][:].opt()])
                    if half == 0:
                        # E prefetch on the Pool queue: first gated-act half
                        # + first down-weight chunks land during half 1.
                        nc.gpsimd.dma_start(
                            grb[0][:],
                            a2a3_out[0][:].rearrange("c (n p) s -> p (c n) s",
                                                     p=P))
                        for idx in range(3):
                            _issue_wd(idx, nc.gpsimd)

            # ======== E: down_proj (seq-shard) + final residual ========
            with (
                tc.tile_pool(name="pE", bufs=2) as pE,
                tc.tile_pool(name="psE", bufs=8, space="PSUM") as psE,
            ):
                nc.gpsimd.dma_start(
                    grb[1][:],
                    a2a3_out[1][:].rearrange("c (n p) s -> p (c n) s", p=P))
                next_issue = [3]

                def consume(idx, ps, start, stop):
                    hf, _d = wd_order[idx]
                    wdc = wd_tiles.pop(idx)
                    nmm = 0
                    for cc in range(NC):
                        for n in range(4):
                            nc.tensor.matmul(
                                ps[:, :SHARD], wdc[:, cc, n, :],
                                grb[hf][:, cc * 4 + n, :],
                                start=(start and nmm == 0),
                                stop=(stop and nmm == 31))
                            nmm += 1
                    if next_issue[0] < len(wd_order):
                        _issue_wd(next_issue[0], nc.sync)
                        next_issue[0] += 1

                def epilogue(dtl, ps):
                    ot = pE.tile([P, SHARD], f32, name="ot")
                    nc.vector.tensor_add(ot[:], ps[:, :SHARD],
                                         x1t[:, dtl, :])
                    nc.sync.dma_start(
                        outt[:].rearrange("(n p) s -> p n s", p=P)[:, dtl],
                        ot[:])

                # two-pass: dtiles 0-7 consume hf0 immediately (prefetched
                # during D), then hf1; dtiles 8-15 run both halves.
                psd = {}
                for d in range(8):
                    psd[d] = psE.tile([P, SBLK], f32, name="eps")
                    consume(d, psd[d], start=True, stop=False)
                for d in range(8):
                    consume(8 + d, psd[d], start=False, stop=True)
                    epilogue(d, psd[d])
                for d in range(8, 16):
                    ps = psE.tile([P, SBLK], f32, name="eps")
                    consume(16 + 2 * (d - 8), ps, start=True, stop=False)
                    consume(16 + 2 * (d - 8) + 1, ps, start=False, stop=True)
                    epilogue(d, ps)
            _pEg_cm.__exit__(None, None, None)
            _pWd_cm.__exit__(None, None, None)
            _pDw_cm.__exit__(None, None, None)
            _pCe_cm.__exit__(None, None, None)
    import concourse.mybir as _mybir
    _split_waits(nc, _mybir)
    return nc


def _host_prep(inputs):
    x = np.ascontiguousarray(inputs["hidden_states"][0])          # [S, D]
    mask = np.ascontiguousarray(inputs["attention_mask"][0, 0])   # [sq, sk]
    maskT = np.ascontiguousarray(mask.T)                          # [sk, sq]
    ln1, ln2 = inputs["ln1_w"], inputs["ln2_w"]
    Wq, Wk, Wv, Wo = inputs["Wq"], inputs["Wk"], inputs["Wv"], inputs["Wo"]
    Wg, Wu, Wd = inputs["Wg"], inputs["Wu"], inputs["Wd"]

    xT = np.ascontiguousarray(x.T)                                # [D, S]
    xT16 = xT.astype(np.float16)
    # packed xt: [j sblk][p][n dchunk][m] = xT[n*128+p, j*512+m]
    xtp = np.ascontiguousarray(
        xT16.reshape(DCH, P, NSB, SBLK).transpose(2, 1, 0, 3))

    inv_freq = 1.0 / (THETA ** (np.arange(0, HD, 2, dtype=np.float32) / HD))
    t = np.arange(S, dtype=np.float32)
    freqs = np.outer(t, inv_freq)
    emb = np.concatenate([freqs, freqs], -1)                      # [S, HD]
    cosT = np.ascontiguousarray(np.cos(emb).T.astype(np.float16))  # [HD, S]
    sinT = np.sin(emb).T.astype(np.float32)
    s2T = sinT.copy()
    s2T[:64] = -s2T[:64]
    s2T = np.ascontiguousarray(s2T.astype(np.float16))

    scale = 1.0 / np.sqrt(HD)
    Wq_f = (ln1[:, None] * Wq * scale).astype(np.float16)   # [D, H*HD]
    Wk_f = (ln1[:, None] * Wk).astype(np.float16)
    Wv_f = (ln1[:, None] * Wv).astype(np.float16)
    Wg_f = (ln2[:, None] * Wg).astype(np.float16)
    Wu_f = (ln2[:, None] * Wu).astype(np.float16)
    Wo16 = Wo.astype(np.float16)                            # [H*HD, D]
    Wd16 = Wd.astype(np.float16)                            # [FF, D]

    # packed wo: [i dtile][p][e chunk][m] = Wo[e*128+p, i*128+m]
    wop = np.ascontiguousarray(
        Wo16.reshape(DCH, P, DCH, P).transpose(2, 1, 0, 3))
    # packed wd: [i][p][f chunk][m] = Wd[f*128+p, i*128+m]
    wdp = np.ascontiguousarray(
        Wd16.reshape(FF // P, P, DCH, P).transpose(2, 1, 0, 3))

    cls = _classify_mask(maskT)
    dmask = np.zeros((4, P, SBLK), np.float32)
    for (ik, jq), c in cls.items():
        if c == "DIAG":
            r = ik - 4 * jq
            assert 0 <= r < 4, "DIAG tile off the ik==4*jq+r band"
            dmask[r] = maskT[ik * P:(ik + 1) * P, jq * SBLK:(jq + 1) * SBLK]
    in_maps = []
    for c in range(NC):
        qsl = slice(2 * P * c, 2 * P * (c + 1))
        kvsl = slice(P * (c // 2), P * (c // 2) + P)
        ffsl = slice(FFSH * c, FFSH * (c + 1))
        ssl = slice(SHARD * c, SHARD * (c + 1))
        wq_c = Wq_f[:, qsl]    # [D, 256]
        wk_c = Wk_f[:, kvsl]   # [D, 128]
        wv_c = Wv_f[:, kvsl]
        wg_c = Wg_f[:, ffsl]   # [D, 1024]
        wu_c = Wu_f[:, ffsl]
        in_maps.append({
            "xt": xtp,
            "xc": np.ascontiguousarray(xT[:, ssl]),
            "cost": cosT,
            "s2t": s2T,
            # [p][n dchunk][cols]
            "wq": np.ascontiguousarray(
                wq_c.reshape(DCH, P, 2 * P).transpose(1, 0, 2)),
            "wk": np.ascontiguousarray(
                wk_c.reshape(DCH, P, P).transpose(1, 0, 2)),
            "wv": np.ascontiguousarray(
                wv_c.reshape(DCH, P, P).transpose(1, 0, 2)),
            "wo": wop,
            # [half][p][n dchunk][ff 512] = Wg_f[n*128+p, half*512+m]
            "wg": np.ascontiguousarray(
                wg_c.reshape(DCH, P, 2, SBLK).transpose(2, 1, 0, 3)),
            "wu": np.ascontiguousarray(
                wu_c.reshape(DCH, P, 2, SBLK).transpose(2, 1, 0, 3)),
            "wd": wdp,
            "maskt": maskT,
            "dmask": dmask,
        })
    return in_maps, cls


def kernel(**inputs):
    from concourse import bass_utils

    in_maps, cls = _host_prep(inputs)
    cls_key = tuple(sorted(cls.items()))
    if cls_key not in _CACHE:
        _CACHE[cls_key] = _build(cls_key, cls)
    nc = _CACHE[cls_key]

    res = bass_utils.run_bass_kernel_spmd(
        nc, in_maps, core_ids=list(range(NC)))
    out = np.empty((S, D), dtype=np.float32)
    for c in range(NC):
        out[SHARD * c:SHARD * (c + 1), :] = res.results[c]["outt"].T
    return out[None]
